# revision 1
# baseline (speedup 1.0000x reference)
"""BitLinear (B=8) tensor-parallel Trainium2 kernel.

Reference computation (see problem):
    gamma = max(max|x|, 1e-5)                  # global over x
    xq    = clip(round(x * 256/gamma), -256, 255)
    beta  = max(mean|W|, 1e-5)                 # global over W
    wq    = clip(round(|W|/beta), -1, 1)       # in {0, 1}
    y     = (xq @ wq.T) * (beta*gamma/256)

Distribution: W rows (out_features) sharded across 8 cores (1376 per core),
x replicated; each core's shard is shipped column-major ([in, out]) so the
quantized weights land directly in the matmul's stationary layout.  gamma and
beta partials are reduced per core and combined with two tiny AllGathers (one
per scalar, so neither chain waits on the other).  The matmul runs in bf16:
xq in [-256,255] and wq in {0,1} are exact in bf16, and products/sums stay
< 2^21 so fp32 PSUM accumulation is exact.

Quantization tricks (all f32-exact, matching jax semantics):
  round-half-even(v) == (v + 1.5*2^23) - 1.5*2^23   (fp32 RNE arithmetic)
  clip(round(v), ..., 255) == round(min(v, 255.49998...))
  wq == (|W| > 0.5*beta)   since round(u)>=1 iff u>0.5, and clip at 1
"""

import numpy as np

# ---- problem constants (hardcoded; kernel.py must be self-contained) ----
B_DIM, S_DIM, I_DIM, O_DIM = 4, 2048, 4096, 11008
N_CORES = 8
O_SHARD = O_DIM // N_CORES          # 1376 out-features per core
T_DIM = B_DIM * S_DIM               # 8192 tokens
TOK_SLICE = T_DIM // N_CORES        # 1024 tokens reduced per core for gamma

EPS = 1e-5
QVAL = 256.0
M_MAGIC = 12582912.0                # 1.5 * 2**23 : fp32 round-to-int magic
CLIP_HI = float(np.nextafter(np.float32(255.5), np.float32(0.0)))


def build_kernel(T=T_DIM, I=I_DIM, O_SH=O_SHARD, n_cores=N_CORES,
                 tok_slice=None, n_total=None):
    """Build + compile the SPMD Bass kernel. Returns the Bacc object.

    Inputs (per core): x [T, I] f32 (replicated), xg [tok_slice, I] f32
    (this core's token slice, for the gamma partial), wt [I, O_SH] f32
    (this core's weight shard, column-major).  Output: y [T, O_SH] f32.
    """
    import concourse.bacc as bacc
    import concourse.mybir as mybir
    import concourse.tile as tile
    from concourse import bass_isa
    from concourse.bass import ts

    if tok_slice is None:
        tok_slice = T // n_cores
    if n_total is None:
        n_total = float(O_DIM) * float(I_DIM)  # mean divisor (full W)

    f32 = mybir.dt.float32
    bf16 = mybir.dt.bfloat16
    Alu = mybir.AluOpType
    Act = mybir.ActivationFunctionType

    KT = I // 128              # k-tiles (contraction)
    ST = T // 128              # token tiles
    GT = tok_slice // 128      # gamma-slice tiles
    # matmul free-dim chunks over the output features (PSUM bank = 512 f32)
    ochunks = []
    off = 0
    while off < O_SH:
        w_ = min(512, O_SH - off)
        ochunks.append((off, w_))
        off += w_

    nc = bacc.Bacc("TRN2", target_bir_lowering=False, debug=False,
                   num_devices=n_cores)

    x_d = nc.dram_tensor("x", [T, I], f32, kind="ExternalInput")
    xg_d = nc.dram_tensor("xg", [tok_slice, I], f32, kind="ExternalInput")
    wt_d = nc.dram_tensor("wt", [I, O_SH], f32, kind="ExternalInput")
    y_d = nc.dram_tensor("y", [T, O_SH], f32, kind="ExternalOutput")
    # collective bounce buffers (internal DRAM; output should be Shared).
    shared = "Shared" if n_cores > 4 else "Local"
    ccw_in = nc.dram_tensor("ccw_in", [1], f32)
    ccw_out = nc.dram_tensor("ccw_out", [n_cores], f32, addr_space=shared)
    ccg_in = nc.dram_tensor("ccg_in", [1], f32)
    ccg_out = nc.dram_tensor("ccg_out", [n_cores], f32, addr_space=shared)

    with tile.TileContext(nc) as tc:
        with (
            tc.tile_pool(name="big", bufs=3) as big_pool,     # [128, I] f32
            tc.tile_pool(name="wtp", bufs=3) as wt_pool,      # [128,O_SH] f32
            tc.tile_pool(name="bfq", bufs=2) as bfq_pool,     # [128, I] bf16
            tc.tile_pool(name="tp", bufs=2) as tp_pool,       # xqT tiles
            tc.tile_pool(name="wres", bufs=1) as wres_pool,   # resident wqT
            tc.tile_pool(name="stat", bufs=1) as stat_pool,   # stats/scalars
            tc.tile_pool(name="yout", bufs=2) as y_pool,      # [128,O_SH] f32
            tc.tile_pool(name="ps", bufs=2, space="PSUM") as ps_pool,
        ):
            wqT = wres_pool.tile([128, KT, O_SH], bf16)
            gmax = stat_pool.tile([128, GT], f32)
            wsum = stat_pool.tile([128, KT], f32)
            redw = stat_pool.tile([128, 1], f32)
            redw2 = stat_pool.tile([128, 1], f32)
            redg = stat_pool.tile([128, 1], f32)
            redg2 = stat_pool.tile([128, 1], f32)
            scw1 = stat_pool.tile([1, n_cores], f32)
            scw = stat_pool.tile([128, n_cores], f32)
            scg1 = stat_pool.tile([1, n_cores], f32)
            scg = stat_pool.tile([128, n_cores], f32)
            scal = stat_pool.tile([128, 8], f32)
            n256 = stat_pool.tile([128, 1], f32)

            # ---- W chain, pass 1: local sum|W| partials ----
            for k in range(KT):
                wt_t = wt_pool.tile([128, O_SH], f32, tag="wtile",
                                    name="wt_t")
                nc.sync.dma_start(wt_t, wt_d[ts(k, 128), :])
                # |w| in place; accum_out gives per-partition sum of |w|
                nc.scalar.activation(wt_t, wt_t, Act.Abs,
                                     accum_out=wsum[:, k:k + 1])
            nc.vector.tensor_reduce(redw, wsum,
                                    axis=mybir.AxisListType.X, op=Alu.add)
            nc.gpsimd.partition_all_reduce(redw2, redw, channels=128,
                                           reduce_op=bass_isa.ReduceOp.add)
            nc.sync.dma_start(ccw_in[:], redw2[0:1, 0:1])
            nc.gpsimd.collective_compute(
                "AllGather", Alu.bypass,
                replica_groups=[list(range(n_cores))],
                ins=[ccw_in.ap()], outs=[ccw_out.ap()])
            nc.sync.dma_start(scw1, ccw_out.ap().rearrange("(a b) -> a b",
                                                           a=1))
            nc.gpsimd.partition_broadcast(scw, scw1)
            nc.vector.tensor_reduce(scal[:, 1:2], scw,
                                    axis=mybir.AxisListType.X, op=Alu.add)
            # mean = sum * fl(1/n)  (DVE has no divide ALU op; <=1ulp vs /n)
            inv_n = float(np.float32(1.0) / np.float32(n_total))
            nc.vector.tensor_scalar_mul(scal[:, 2:3], scal[:, 1:2], inv_n)
            nc.vector.tensor_scalar_max(scal[:, 2:3], scal[:, 2:3], EPS)
            # half_beta = 0.5*beta (exact)
            nc.vector.tensor_scalar_mul(scal[:, 4:5], scal[:, 2:3], 0.5)

            # ---- gamma chain: local max|x| over this core's token slice ----
            for t in range(GT):
                xg_t = big_pool.tile([128, I], f32, tag="bigtile",
                                     name="xg_t")
                nc.sync.dma_start(xg_t, xg_d[ts(t, 128), :])
                nc.vector.tensor_reduce(
                    gmax[:, t:t + 1], xg_t, axis=mybir.AxisListType.X,
                    op=Alu.max, apply_absolute_value=True)
            nc.vector.tensor_reduce(redg, gmax,
                                    axis=mybir.AxisListType.X, op=Alu.max)
            nc.gpsimd.partition_all_reduce(redg2, redg, channels=128,
                                           reduce_op=bass_isa.ReduceOp.max)
            nc.sync.dma_start(ccg_in[:], redg2[0:1, 0:1])
            nc.gpsimd.collective_compute(
                "AllGather", Alu.bypass,
                replica_groups=[list(range(n_cores))],
                ins=[ccg_in.ap()], outs=[ccg_out.ap()])
            nc.sync.dma_start(scg1, ccg_out.ap().rearrange("(a b) -> a b",
                                                           a=1))
            nc.gpsimd.partition_broadcast(scg, scg1)
            nc.vector.tensor_reduce(scal[:, 0:1], scg,
                                    axis=mybir.AxisListType.X, op=Alu.max)
            nc.vector.tensor_scalar_max(scal[:, 0:1], scal[:, 0:1], EPS)
            # s_x = 256/gamma via hw reciprocal; x256 is an exact pow2 scale,
            # so if reciprocal is correctly rounded this equals fl(256/gamma)
            nc.vector.reciprocal(n256, scal[:, 0:1])
            nc.vector.tensor_scalar_mul(scal[:, 3:4], n256, QVAL)
            # c_out = (beta*gamma)/256 : fp32 mult then exact pow2 scale
            nc.vector.tensor_tensor(scal[:, 5:6], scal[:, 2:3], scal[:, 0:1],
                                    op=Alu.mult)
            nc.vector.tensor_scalar_mul(scal[:, 5:6], scal[:, 5:6],
                                        1.0 / 256.0)

            # prefetch the first x tiles so quantization can start the
            # moment gamma lands (they ride the DMA queue ahead of pass 2)
            pre_x = []
            for st in range(2):
                x_t = big_pool.tile([128, I], f32, tag="bigtile",
                                    name="x_pre")
                nc.sync.dma_start(x_t, x_d[ts(st, 128), :])
                pre_x.append(x_t)

            # ---- W chain, pass 2: quantize straight into wqT layout ----
            for k in range(KT):
                wt_t = wt_pool.tile([128, O_SH], f32, tag="wtile",
                                    name="wt_t2")
                nc.sync.dma_start(wt_t, wt_d[ts(k, 128), :])
                nc.scalar.activation(wt_t, wt_t, Act.Abs)
                nc.vector.tensor_scalar(wqT[:, k, :], wt_t, scal[:, 4:5],
                                        None, op0=Alu.is_gt)

            # ---- main loop: stream tokens, quantize, transpose, matmul ----
            for st in range(ST):
                if st < len(pre_x):
                    x_t = pre_x[st]
                else:
                    x_t = big_pool.tile([128, I], f32, tag="bigtile",
                                        name="x_t")
                    nc.sync.dma_start(x_t, x_d[ts(st, 128), :])
                # v = min(x*s_x, 255.4999...)   (in place, f32)
                nc.vector.tensor_scalar(x_t, x_t, scal[:, 3:4], CLIP_HI,
                                        op0=Alu.mult, op1=Alu.min)
                # round-half-even via +M -M; output bf16 (exact small ints)
                xq_t = bfq_pool.tile([128, I], bf16, tag="bfqtile",
                                     name="xq_t")
                nc.vector.tensor_scalar(xq_t, x_t, M_MAGIC, M_MAGIC,
                                        op0=Alu.add, op1=Alu.subtract)
                xqT_t = tp_pool.tile([128, KT, 128], bf16, name="xqT_t")
                nc.scalar.dma_start(xqT_t, xq_t, transpose=True)

                ps_t = ps_pool.tile([128, O_SH], f32, name="ps_t")
                for (off, width) in ochunks:
                    for k in range(KT):
                        nc.tensor.matmul(
                            ps_t[:, off:off + width],
                            xqT_t[:, k, :],
                            wqT[:, k, off:off + width],
                            start=(k == 0), stop=(k == KT - 1))
                y_t = y_pool.tile([128, O_SH], f32, name="y_t")
                nc.scalar.activation(y_t, ps_t, Act.Copy,
                                     scale=scal[:, 5:6])
                nc.gpsimd.dma_start(y_d[ts(st, 128), :], y_t)

    nc.compile()
    return nc


_CACHED_NC = None


def _get_nc():
    global _CACHED_NC
    if _CACHED_NC is None:
        _CACHED_NC = build_kernel()
    return _CACHED_NC


def shard_inputs(x, weight):
    """Host-side sharding/marshalling: full inputs -> per-core input maps."""
    x2 = np.ascontiguousarray(x.reshape(T_DIM, I_DIM).astype(np.float32,
                                                             copy=False))
    weight = weight.astype(np.float32, copy=False)
    in_maps = []
    for c in range(N_CORES):
        in_maps.append({
            "x": x2,
            "xg": np.ascontiguousarray(
                x2[c * TOK_SLICE:(c + 1) * TOK_SLICE]),
            "wt": np.ascontiguousarray(
                weight[c * O_SHARD:(c + 1) * O_SHARD].T),
        })
    return in_maps


def unshard_output(results):
    """Per-core y [T, O_SHARD] -> full y [B, S, O]."""
    parts = [results[c]["y"] for c in range(N_CORES)]
    return np.concatenate(parts, axis=1).reshape(B_DIM, S_DIM, O_DIM)


def run_on_cores(x, weight, trace=False):
    from concourse.bass_utils import run_bass_kernel_spmd
    nc = _get_nc()
    in_maps = shard_inputs(x, weight)
    res = run_bass_kernel_spmd(nc, in_maps, core_ids=list(range(N_CORES)),
                               trace=trace)
    return res


def kernel(x, weight):
    res = run_on_cores(x, weight, trace=False)
    return unshard_output(res.results)

